# revision 27
# baseline (speedup 1.0000x reference)
"""RNN wavefunction (2-layer GRU, H=128, 64 steps, B=65536) on 8 TRN2 cores.

Strategy (pure data parallelism over batch):
  - Each core handles BC = 8192 samples, in 16 chunks of 512; chunks run in
    pairs (A, B) as two independent software pipelines with chunk-dedicated
    PSUM slots (issue order l0(A), l0(B), l1(A), l1(B) keeps pool-slot
    affinity per chunk, removing cross-chunk false deps).
  - Phase-1 state (h, gates, elementwise) in fp16: DVE gets 2x packing;
    gate matmuls in fp16 (1 cyc/row).  Head/logit path stays f32r/f32
    (phase accumulation amplifies systematic logit error 64x).
  - Head matmuls are issued one step late so they don't block the next
    step's layer-0 matmuls in the PE FIFO.
  - n-gate: u = gi_n + r*gh_n via identity-matmul PSUM rebuild (sets
    has_written so gi_n can accumulate).
  - Phase 2 (softmax/sqrt/softsign/mask/normalize, log-amp + phase sums via
    ones-matmul) runs per pass directly from the head PSUM stage in SBUF --
    no DRAM scratch round trip -- overlapping the next pass's recurrence.
"""
import os
import numpy as np
from contextlib import ExitStack

import concourse.bacc as bacc
import concourse.tile as tile
import concourse.mybir as mybir
from concourse.bass_utils import run_bass_kernel_spmd

F32 = mybir.dt.float32
F32R = mybir.dt.float32r
F16 = mybir.dt.float16
AF = mybir.ActivationFunctionType
OP = mybir.AluOpType

SORB = 64
NELE = 32
H = 128
B = 65536
NCORES = 8
BC = B // NCORES          # 8192 samples per core
CH = 512                  # chunk size (samples per PSUM bank row)
NCHUNK = BC // CH         # 16
GS = 22                   # steps per partition-group (3 groups: 22/22/20)
SLAB = 2 * CH             # phase-2 columns per pass (one chunk pair)
PI = float(np.pi)
RND = 12582912.0          # 1.5 * 2**23, round-to-nearest trick

_CACHE = {}


def build(dbg=False):
    nc = bacc.Bacc("TRN2", target_bir_lowering=False, debug=False)

    # ---- per-core inputs ----
    bits_gr = nc.declare_dram_parameter("bits_gr", [3, GS, BC], F16, isOutput=False)
    bits_t = nc.declare_dram_parameter("bits_t", [SORB, BC], F32, isOutput=False)
    umask = nc.declare_dram_parameter("umask", [SORB, BC], F32, isOutput=False)
    omask = nc.declare_dram_parameter("omask", [SORB, BC], F32, isOutput=False)
    mselm = nc.declare_dram_parameter("mselm", [SORB, BC], F32, isOutput=False)
    # ---- replicated weights ----
    ghw_p = nc.declare_dram_parameter("ghw", [128, 9 * H], F16, isOutput=False)
    gi0w_p = nc.declare_dram_parameter("gi0w", [2, 3 * H], F16, isOutput=False)
    headw_p = nc.declare_dram_parameter("headw", [128, 192], F16, isOutput=False)
    blin_p = nc.declare_dram_parameter("blin", [128, 1], F32, isOutput=False)
    ident_p = nc.declare_dram_parameter("ident", [128, 128], F16, isOutput=False)
    ones_p = nc.declare_dram_parameter("ones64", [SORB, 1], F32R, isOutput=False)
    obones_p = nc.declare_dram_parameter("obones", [3, GS * CH], F16, isOutput=False)
    # ---- output ----
    out_p = nc.declare_dram_parameter("o", [2, BC], F32, isOutput=True)

    with tile.TileContext(nc) as tc, ExitStack() as octx:
        ppool = octx.enter_context(tc.tile_pool(name="persist", bufs=1))
        gpool = octx.enter_context(tc.tile_pool(name="gates", bufs=2, space="PSUM"))
        hdpool = octx.enter_context(tc.tile_pool(name="head", bufs=2, space="PSUM"))
        hpool = octx.enter_context(tc.tile_pool(name="hstate", bufs=10))
        work = octx.enter_context(tc.tile_pool(name="work", bufs=4))
        obtpool = octx.enter_context(tc.tile_pool(name="obt", bufs=2))
        p2 = octx.enter_context(tc.tile_pool(name="p2", bufs=1))
        p2in = octx.enter_context(tc.tile_pool(name="p2in", bufs=2))

        # --- persistent small tensors + weights ---
        blin = ppool.tile([128, 1], F32)
        ones64 = ppool.tile([SORB, 1], F32R)
        nc.sync.dma_start(blin[:], blin_p[:])
        nc.sync.dma_start(ones64[:], ones_p[:])
        ghw = ppool.tile([128, 9 * H], F16)
        nc.sync.dma_start(ghw[:], ghw_p[:])
        headw = ppool.tile([128, 192], F16)
        nc.sync.dma_start(headw[:], headw_p[:])
        gi0w = ppool.tile([66, 3 * H], F16)
        for g in range(3):
            nc.sync.dma_start(gi0w[32 * g:32 * g + 2, :], gi0w_p[:])
        ident = ppool.tile([128, 128], F16)
        nc.sync.dma_start(ident[:], ident_p[:])
        halfpi = p2.tile([1, 1], F32, tag="halfpi")
        nc.gpsimd.memset(halfpi[:], 0.0)
        zh = ppool.tile([128, CH], F16)   # shared read-only zero h-state
        nc.vector.memset(zh[:], 0.0)

        nchunk_run = int(os.environ.get("K_NCHUNK", NCHUNK))

        def gates_rz_mms(c, st, wofs, gi0_g, gi0_rhs):
            """The 4 r/z gate matmuls (accumulated gh+gi pairs)."""
            h_prev, rhs_in = st["h"][c], st["rhs"][c]
            rz = gpool.tile([128, 2 * CH], F32, tag="rz")
            if rhs_in is None:
                ghofs = wofs
                nc.tensor.matmul(rz[:, 0:CH], ghw[:, ghofs:ghofs + H], h_prev[:], start=True, stop=False)
                nc.tensor.matmul(rz[:, 0:CH], gi0w[gi0_g:gi0_g + 2, 0:H], gi0_rhs, start=False, stop=True)
                nc.tensor.matmul(rz[:, CH:], ghw[:, ghofs + H:ghofs + 2 * H], h_prev[:], start=True, stop=False)
                nc.tensor.matmul(rz[:, CH:], gi0w[gi0_g:gi0_g + 2, H:2 * H], gi0_rhs, start=False, stop=True)
            else:
                giofs, ghofs = wofs, wofs + 3 * H
                nc.tensor.matmul(rz[:, 0:CH], ghw[:, giofs:giofs + H], rhs_in[:], start=True, stop=False)
                nc.tensor.matmul(rz[:, 0:CH], ghw[:, ghofs:ghofs + H], h_prev[:], start=False, stop=True)
                nc.tensor.matmul(rz[:, CH:], ghw[:, giofs + H:giofs + 2 * H], rhs_in[:], start=True, stop=False)
                nc.tensor.matmul(rz[:, CH:], ghw[:, ghofs + H:ghofs + 2 * H], h_prev[:], start=False, stop=True)
            st["rz"][c] = rz

        def gates_nx_mm(c, st, wofs):
            """The gh_n matmul (its own psum bank; slot frees at tt-read)."""
            h_prev = st["h"][c]
            ghofs = wofs if st["rhs"][c] is None else wofs + 3 * H
            nx = gpool.tile([128, CH], F32, tag="nx")
            nc.tensor.matmul(nx[:], ghw[:, ghofs + 2 * H:ghofs + 3 * H], h_prev[:], start=True, stop=True)
            st["nx"][c] = nx

        def layer_sig(st):
            """Combined r+z sigmoid per chunk (one ACT op over both banks)."""
            st["rzs"] = {}
            for c in st["cc"]:
                rzs = work.tile([128, 2 * CH], F16, tag="rzs", name="rzs", bufs=3)
                nc.scalar.activation(rzs[:], st["rz"][c][:], AF.Sigmoid)
                st["rzs"][c] = rzs

        def layer_tt(st):
            st["tt"] = {}
            for c in st["cc"]:
                tt = work.tile([128, CH], F16, tag="tt", name="tt")
                nc.vector.tensor_mul(tt[:], st["rzs"][c][:, 0:CH], st["nx"][c][:])
                st["tt"][c] = tt

        def layer_uacc(st, wofs, gi0_g, gi0_rhs_f):
            for c in st["cc"]:
                nx = st["nx"][c]
                nc.tensor.matmul(nx[:], ident[:], st["tt"][c][:], start=True, stop=False,
                                 skip_group_check=True)
                if st["rhs"][c] is None:
                    nc.tensor.matmul(nx[:], gi0w[gi0_g:gi0_g + 2, 2 * H:3 * H], gi0_rhs_f(c),
                                     start=False, stop=True, skip_group_check=True)
                else:
                    nc.tensor.matmul(nx[:], ghw[:, wofs + 2 * H:wofs + 3 * H], st["rhs"][c][:],
                                     start=False, stop=True, skip_group_check=True)

        def layer_tanh(st):
            st["ns"] = {}
            for c in st["cc"]:
                ns = work.tile([128, CH], F16, tag="ns", name="ns")
                nc.scalar.activation(ns[:], st["nx"][c][:], AF.Tanh)
                st["ns"][c] = ns

        def layer_ds(st):
            HC = CH // 2
            st["ds"] = {}
            for c in st["cc"]:
                # ds split: low half on Pool, high half on DVE (halves path time)
                ds = work.tile([128, CH], F16, tag="ds", name="ds")
                nc.gpsimd.tensor_sub(ds[:, 0:HC], st["h"][c][:, 0:HC], st["ns"][c][:, 0:HC])
                nc.vector.tensor_sub(ds[:, HC:], st["h"][c][:, HC:], st["ns"][c][:, HC:])
                st["ds"][c] = ds

        def layer_hnew(st):
            for c in st["cc"]:
                es = work.tile([128, CH], F16, tag="es")
                nc.vector.tensor_mul(es[:], st["rzs"][c][:, CH:], st["ds"][c][:])
                h_new = hpool.tile([128, CH], F16, tag="h")
                nc.vector.tensor_add(h_new[:], st["ns"][c][:], es[:])
                st["h"][c] = h_new

        for p in range(0, nchunk_run, 2):
            cc = [c for c in (p, p + 1) if c < nchunk_run]
            sl = slice(p * CH, (p + len(cc)) * CH)
            ncol = len(cc) * CH

            # phase-2 inputs for this pass: prefetch at pass start
            bits_s = p2in.tile([SORB, SLAB], F32, tag="bits_s")
            u_s = p2in.tile([SORB, SLAB], F32, tag="u_s")
            o_s = p2in.tile([SORB, SLAB], F32, tag="o_s")
            m_s = p2in.tile([SORB, SLAB], F32, tag="m_s")
            nc.sync.dma_start(bits_s[:, 0:ncol], bits_t[:, sl])
            nc.sync.dma_start(u_s[:, 0:ncol], umask[:, sl])
            nc.sync.dma_start(o_s[:, 0:ncol], omask[:, sl])
            nc.sync.dma_start(m_s[:, 0:ncol], mselm[:, sl])

            obt, h0, h1, heads = {}, {}, {}, {}
            for c in cc:
                ob = obtpool.tile([66, GS * CH], F16, tag="obts")
                for g in range(3):
                    nc.sync.dma_start(ob[32 * g:32 * g + 1, :], obones_p[g:g + 1, :])
                    n_t = GS if g < 2 else SORB - 2 * GS
                    nc.sync.dma_start(ob[32 * g + 1:32 * g + 2, 0:n_t * CH],
                                      bits_gr[g, 0:n_t, c * CH:(c + 1) * CH])
                obt[c] = ob
                h0[c] = zh
                h1[c] = zh
                heads[c] = hdpool.tile([128, CH], F32, tag="hd", name=f"hd{c%2}")

            st0 = {"cc": cc, "h": h0, "rz": {}, "nx": {}, "rhs": {c: None for c in cc},
                   "h1sav": {}}
            st1 = {"cc": cc, "h": h1, "rz": {}, "nx": {}, "rhs": {}, "h1sav": {}}
            # Software-pipeline the two layers with a one-step delay: at tick
            # tau, layer 0 computes h0(tau) while layer 1 computes h1(tau-1)
            # from LAST tick's h0 -- the two chains run fully in parallel and
            # the tick is issued as one flat engine-ordered block: all 20 gate
            # matmuls (which depend only on last-tick state) go first, so the
            # PE never stalls waiting for this tick's sigmoids.
            # Head mm for step s is issued at tick s+2 (h1(s) ready at s+1).
            h0pipe = []
            h1hist = {}   # h1 by step, for delayed head mms
            for tau in range(SORB + 1):
                do0, do1 = tau < SORB, tau >= 1
                if do0:
                    g, j = divmod(tau, GS)
                    base = 32 * g

                    def gi0_rhs(c, _base=base, _j=j):
                        return obt[c][_base:_base + 2, _j * CH:(_j + 1) * CH]

                    for c in cc:
                        gates_rz_mms(c, st0, 0, base, gi0_rhs(c))
                        gates_nx_mm(c, st0, 0)
                    hs = tau - 2
                    if hs >= 0:
                        hv = h1hist.pop(hs)
                        for c in cc:
                            nc.tensor.matmul(heads[c][:], headw[:, 64 - hs:192 - hs],
                                             hv[c][:], start=(hs == 0), stop=False,
                                             skip_group_check=True)
                    layer_sig(st0)
                    layer_tt(st0)
                    layer_uacc(st0, 0, base, gi0_rhs)
                    layer_tanh(st0)
                    layer_ds(st0)
                    layer_hnew(st0)
                    h0pipe.append(dict(st0["h"]))
                if do1:
                    st1["rhs"] = h0pipe.pop(0)
                    for c in cc:
                        gates_rz_mms(c, st1, 3 * H, 0, None)
                        gates_nx_mm(c, st1, 3 * H)
                    layer_sig(st1)
                    layer_tt(st1)
                    layer_uacc(st1, 3 * H, 0, None)
                    layer_tanh(st1)
                    layer_ds(st1)
                    layer_hnew(st1)
                    h1hist[tau - 1] = dict(st1["h"])
            # tail heads: steps 62 and 63
            for hs in (62, 63):
                hv = h1hist.pop(hs)
                for c in cc:
                    nc.tensor.matmul(heads[c][:], headw[:, 64 - hs:192 - hs], hv[c][:],
                                     start=(hs == 0), stop=(hs == 63), skip_group_check=True)

            # ---------------- phase 2 for this pass ----------------
            hstage = p2in.tile([128, SLAB], F32, tag="hstage")
            for i, c in enumerate(cc):
                nc.scalar.activation(hstage[:, i * CH:(i + 1) * CH], heads[c][:],
                                     AF.Identity, bias=blin[:, 0:1])
            D = hstage[0:64, :]
            # L0 lives on partitions 64-127; engines can't cross partition
            # bases, so DMA it down to a partition-0-based tile (off-path)
            l0t = p2in.tile([SORB, SLAB], F32, tag="l0t")
            nc.sync.dma_start(l0t[:], hstage[64:128, :])
            L0 = l0t[:]

            lsel = p2.tile([SORB, SLAB], F32, tag="lsel")
            nc.vector.tensor_mul(lsel[:], bits_s[:], D)
            nc.gpsimd.tensor_add(lsel[:], lsel[:], L0)
            absl = p2.tile([SORB, SLAB], F32, tag="absl")
            nc.scalar.activation(absl[:], lsel[:], AF.Abs)
            nc.vector.tensor_scalar_add(absl[:], absl[:], 1.0)
            rabs = absl
            nc.vector.reciprocal(rabs[:], absl[:])
            phsel = p2.tile([SORB, SLAB], F32R, tag="phsel")
            nc.vector.scalar_tensor_tensor(phsel[:], lsel[:], PI, rabs[:],
                                           op0=OP.mult, op1=OP.mult)

            s1 = p2.tile([SORB, SLAB], F32, tag="s1")
            nc.scalar.activation(s1[:], D, AF.Sigmoid)
            s0 = p2.tile([SORB, SLAB], F32, tag="s0")
            nc.vector.tensor_scalar(s0[:], s1[:], -1.0, 1.0, op0=OP.mult, op1=OP.add)
            a1 = p2.tile([SORB, SLAB], F32, tag="a1")
            nc.scalar.activation(a1[:], s1[:], AF.Sqrt)
            a0 = p2.tile([SORB, SLAB], F32, tag="a0")
            nc.scalar.activation(a0[:], s0[:], AF.Sqrt)
            asel = p2.tile([SORB, SLAB], F32, tag="asel")
            nc.gpsimd.tensor_sub(asel[:], a1[:], a0[:])
            nc.vector.tensor_mul(asel[:], bits_s[:], asel[:])
            nc.gpsimd.tensor_add(asel[:], asel[:], a0[:])

            nrm2 = p2.tile([SORB, SLAB], F32, tag="nrm2")
            nc.vector.tensor_mul(nrm2[:], u_s[:], s0[:])
            m2t = lsel
            nc.gpsimd.tensor_mul(m2t[:], o_s[:], s1[:])
            nc.vector.tensor_add(nrm2[:], nrm2[:], m2t[:])
            nc.scalar.activation(nrm2[:], nrm2[:], AF.Sqrt)
            nc.vector.tensor_scalar_max(nrm2[:], nrm2[:], 1e-12)
            rn = nrm2
            nc.vector.reciprocal(rn[:], nrm2[:])

            ampsel = asel
            nc.gpsimd.tensor_mul(ampsel[:], asel[:], m_s[:])
            nc.vector.tensor_mul(ampsel[:], ampsel[:], rn[:])
            nc.vector.tensor_scalar_max(ampsel[:], ampsel[:], 1e-30)
            lna = p2.tile([SORB, SLAB], F32R, tag="lna")
            nc.scalar.activation(lna[:], ampsel[:], AF.Ln)

            # reductions over the 64 steps via ones-matmul into spare psum
            phl = gpool.tile([128, 2 * CH], F32, tag="rz")
            lnl_a = gpool.tile([128, CH], F32, tag="nx")
            lnl_b = gpool.tile([128, CH], F32, tag="nx")
            nc.tensor.matmul(phl[0:1, 0:CH], ones64[:], phsel[:, 0:CH], start=True, stop=True)
            nc.tensor.matmul(phl[0:1, CH:2 * CH], ones64[:], phsel[:, CH:2 * CH], start=True, stop=True)
            nc.tensor.matmul(lnl_a[0:1, :], ones64[:], lna[:, 0:CH], start=True, stop=True)
            nc.tensor.matmul(lnl_b[0:1, :], ones64[:], lna[:, CH:2 * CH], start=True, stop=True)

            amp = p2.tile([1, SLAB], F32, tag="amp")
            nc.scalar.activation(amp[:, 0:CH], lnl_a[0:1, :], AF.Exp)
            nc.scalar.activation(amp[:, CH:2 * CH], lnl_b[0:1, :], AF.Exp)
            y = p2.tile([1, SLAB], F32, tag="y")
            nc.vector.tensor_scalar_mul(y[:], phl[0:1, 0:SLAB], 1.0 / (2.0 * PI))
            fr = p2.tile([1, SLAB], F32, tag="fr")
            nc.vector.tensor_scalar(fr[:], y[:], RND, RND, op0=OP.add, op1=OP.subtract)
            nc.vector.tensor_sub(fr[:], y[:], fr[:])
            yc = p2.tile([1, SLAB], F32, tag="yc")
            nc.vector.tensor_scalar_add(yc[:], y[:], 0.25)
            frc = p2.tile([1, SLAB], F32, tag="frc")
            nc.vector.tensor_scalar(frc[:], yc[:], RND, RND, op0=OP.add, op1=OP.subtract)
            nc.vector.tensor_sub(frc[:], yc[:], frc[:])
            sinv = p2.tile([1, SLAB], F32, tag="sinv")
            nc.scalar.activation(sinv[:], fr[:], AF.Sin, scale=2.0 * PI)
            cosv = p2.tile([1, SLAB], F32, tag="cosv")
            nc.scalar.activation(cosv[:], frc[:], AF.Sin, scale=2.0 * PI)
            re = p2.tile([1, SLAB], F32, tag="re")
            im = p2.tile([1, SLAB], F32, tag="im")
            nc.vector.tensor_mul(re[:], amp[:], cosv[:])
            nc.vector.tensor_mul(im[:], amp[:], sinv[:])
            nc.sync.dma_start(out_p[0:1, sl], re[:, 0:ncol])
            nc.sync.dma_start(out_p[1:2, sl], im[:, 0:ncol])
    nc.compile()
    return nc


def _obones():
    ob = np.ones((3, GS * CH), np.float16)
    ob[0, 0:CH] = 0.0   # step t=0: x0 is all-zeros, kill the c0 column too
    return ob


def _prep(x, w_ih0, w_hh0, w_ih1, w_hh1, w_lin, b_lin):
    """CPU-side preprocessing. Returns (shared weight arrays, per-core arrays)."""
    x = np.asarray(x)
    bits = ((x.astype(np.int32) + 1) // 2).astype(np.int32)  # (B, 64)
    bits_T = np.ascontiguousarray(bits.T)                    # (64, B)
    alpha = NELE // 2  # 16

    # prefix counts BEFORE step t (masking uses counts from previous steps)
    bf = bits_T.astype(np.int64)
    even_mask = (np.arange(SORB) % 2 == 0)[:, None]
    up_incr = np.where(even_mask, bf, 0)
    dn_incr = np.where(even_mask, 0, bf)
    num_up = np.cumsum(up_incr, axis=0) - up_incr       # exclusive prefix
    num_dn = np.cumsum(dn_incr, axis=0) - dn_incr
    num = np.where(even_mask, num_up, num_dn)           # (64, B)
    tvec = np.arange(SORB)[:, None]
    lower = tvec // 2 - (SORB // 2 - alpha)             # t//2 - 16
    occ = (num < alpha).astype(np.float32)
    unocc = (num > lower).astype(np.float32)
    pre = tvec < alpha                                   # t < 16: no masking
    occ = np.where(pre, 1.0, occ).astype(np.float32)
    unocc = np.where(pre, 1.0, unocc).astype(np.float32)
    msel = np.where(bits_T == 1, occ, unocc).astype(np.float32)

    bits_Tf = bits_T.astype(np.float32)

    # grouped bits for the gi0 rank-2 matmuls. Layer-0 input at step t is
    # one_hot(b_{t-1}) (zeros at t=0), so shift by one step.
    bits_prev = np.concatenate([np.zeros((1, B), np.float32), bits_Tf[:-1]], axis=0)
    bits_gr = np.zeros((3, GS, B), np.float16)
    for g in range(3):
        n_t = GS if g < 2 else SORB - 2 * GS
        bits_gr[g, 0:n_t] = bits_prev[g * GS:g * GS + n_t]

    def lhsT(w):
        return np.ascontiguousarray(w.T)

    ghw = np.concatenate([
        lhsT(w_hh0[0:H]), lhsT(w_hh0[H:2 * H]), lhsT(w_hh0[2 * H:3 * H]),
        lhsT(w_ih1[0:H]), lhsT(w_ih1[H:2 * H]), lhsT(w_ih1[2 * H:3 * H]),
        lhsT(w_hh1[0:H]), lhsT(w_hh1[H:2 * H]), lhsT(w_hh1[2 * H:3 * H]),
    ], axis=1).astype(np.float16)                       # (128, 9*128)

    c0 = w_ih0[:, 0]
    dc = w_ih0[:, 1] - w_ih0[:, 0]
    gi0w = np.stack([c0, dc], axis=0).astype(np.float16)  # (2, 384)

    headw = np.zeros((128, 192), np.float16)
    wd = (w_lin[1] - w_lin[0]).astype(np.float32)
    w0 = w_lin[0].astype(np.float32)
    headw[:, 64] = wd
    headw[:, 128] = w0

    blin = np.zeros((128, 1), np.float32)
    blin[0:64, 0] = b_lin[1] - b_lin[0]
    blin[64:128, 0] = b_lin[0]

    ident = np.eye(128, dtype=np.float16)
    ones64 = np.ones((SORB, 1), np.float32)

    shared = dict(ghw=ghw, gi0w=gi0w, headw=headw, blin=blin, ident=ident,
                  ones64=ones64, obones=_obones())
    per_core = []
    for k in range(NCORES):
        cs = slice(k * BC, (k + 1) * BC)
        per_core.append(dict(
            bits_gr=np.ascontiguousarray(bits_gr[:, :, cs]),
            bits_t=np.ascontiguousarray(bits_Tf[:, cs]),
            umask=np.ascontiguousarray(unocc[:, cs]),
            omask=np.ascontiguousarray(occ[:, cs]),
            mselm=np.ascontiguousarray(msel[:, cs]),
        ))
    return shared, per_core


def kernel(x, w_ih0, w_hh0, w_ih1, w_hh1, w_lin, b_lin, _trace=False):
    shared, per_core = _prep(np.asarray(x), np.asarray(w_ih0), np.asarray(w_hh0),
                             np.asarray(w_ih1), np.asarray(w_hh1),
                             np.asarray(w_lin), np.asarray(b_lin))
    if "nc" not in _CACHE:
        _CACHE["nc"] = build()
    nc = _CACHE["nc"]
    in_maps = [{**shared, **pc} for pc in per_core]
    res = run_bass_kernel_spmd(nc, in_maps, list(range(NCORES)), trace=_trace)
    _CACHE["last_exec_time_ns"] = res.exec_time_ns
    out = np.empty((2, B), np.float32)
    for k in range(NCORES):
        out[:, k * BC:(k + 1) * BC] = res.results[k]["o"]
    return out
